# revision 2
# baseline (speedup 1.0000x reference)
"""Causal Performer attention on 8 trn2 NeuronCores (bf16, pipelined).

Sharding: core c handles batch b = c // 4 and head-group hg = c % 4
(3 of the 12 heads). Each core:
  1. computes the qkv projection for its 3 heads (576 of 2304 rows),
  2. runs the causal linear-attention scan in chunked form: intra-chunk
     masked (K'Q') scores plus an inter-chunk (F,D) running state,
     updated/applied every 128-token chunk with the 3 heads interleaved
     inside each chunk step,
  3. projects its 192 channels into a partial y in [t, c]-major layout
     whose 128-row chunks stream out over DMA as they are computed.
The host upcasts and sums the four partial bf16 (T, C) projections per
batch (pure gather/unshard work).

All matmul operands are bf16; PSUM accumulation stays fp32 (measured
rel err vs the fp32 reference: 1.03e-2, gate 2e-2). bf16 runs every
matmul at 1 cycle/row (small-N fp32 matmuls cost 4) and halves the
input DMA stream, which paces the front-end.

Key structures (emission order is the per-engine execution order):
  - q'^T/K'^T = exp([P; -0.5*ones]^T [q; q^2]) via one stacked matmul
    per half; q' and k' share one [128,512] PSUM tile (q rows 0:64,
    k rows 64:128) so a single ACT Exp pass serves both; Pool copies
    the k half to a base-0 tile for the scores stationary (PE operands
    must share a base partition). Squares run on DVE from bf16 rows.
  - The front-end is software-pipelined: head h+1's qkv matmuls run
    while head h's stk copies / squares / Exp chain drains; vdir
    (V in natural [t,d] layout) fills the remaining PE gaps.
  - The scan state accumulates in SBUF: each chunk's update lands in a
    fresh PSUM delta tile (3 per-head groups that open and close within
    the chunk -- re-opening a stopped PSUM accumulation group loses its
    contents) and one DVE tensor_add folds it into the running bf16
    state while producing the next apply operand.
  - EPS rides on the numerator/denominator tile via K=1 matmuls
    (ones[1,chunk] x [0..0,EPS] per head) so the division needs no
    extra scalar adds; scores+mask are pipelined two chunks ahead
    (mask multiply alternates DVE direct and ACT-copy + Pool-mul);
    division lags one chunk, off the state chain.
  - V for the 3 heads lives interleaved [v0|1|v1|1|v2|1] in one [S,195]
    tile per chunk (one strided DVE copy; ones columns memset once);
    the 3 heads' scores / numden share PSUM tiles (one mask op, one
    strided reciprocal).
  - Per chunk, och is transposed ([t,ch] -> [ch,t]) and used as the
    yproj stationary with wpt moving, yielding y[t-chunk, 768] directly;
    the row-chunk DMAs out immediately, so there is no output tail.
"""

import numpy as np

import concourse.bacc as bacc
import concourse.bass as bass
import concourse.mybir as mybir
from concourse import tile
from concourse.bass_utils import run_bass_kernel_spmd

B, T, C = 2, 1024, 768
H, D, F = 12, 64, 64
HPC = 3  # heads per core
S = 128  # scan chunk length
NCH = T // S  # 8 chunks
CP = HPC * D  # 192 channels per core
N_CORES = 8
KT = C // 128  # 6 contraction tiles for the qkv matmul
EPS_SCALED = float(F) * 1e-6  # compensates dropping 1/sqrt(F) on q', k'
E = D + 1  # 65: [V | 1] block width

FP32 = mybir.dt.float32
BF16 = mybir.dt.bfloat16
EXP = mybir.ActivationFunctionType.Exp
COPY = mybir.ActivationFunctionType.Copy


def build_program(n_iters=1):
    nc = bacc.Bacc(
        "TRN2", target_bir_lowering=False, debug=False, num_devices=N_CORES
    )
    xt = nc.dram_tensor("xt", [C, T], BF16, kind="ExternalInput").ap()
    wt = nc.dram_tensor("wt", [C, 3 * CP], BF16, kind="ExternalInput").ap()
    wpt = nc.dram_tensor("wpt", [CP, C], BF16, kind="ExternalInput").ap()
    projext = nc.dram_tensor("projext", [2 * D, F], BF16, kind="ExternalInput").ap()
    mask3 = nc.dram_tensor("mask3", [S, 3 * S], BF16, kind="ExternalInput").ap()
    ident = nc.dram_tensor("ident", [128, 128], BF16, kind="ExternalInput").ap()
    y = nc.dram_tensor("y", [T, C], BF16, kind="ExternalOutput").ap()

    from contextlib import ExitStack

    with tile.TileContext(nc) as tc:
        for _ in range(n_iters):
            with ExitStack() as ctx:
                _body(ctx, tc, xt, wt, wpt, projext, mask3, ident, y)
    nc.compile()
    return nc


def _body(ctx, tc, xt, wt, wpt, projext, mask3, ident, y):
    nc = tc.nc

    const = ctx.enter_context(tc.tile_pool(name="const", bufs=1))

    # xt + late consts on the SP queue; wt/wpt on the ACT queue. xt0 and
    # wt0 are issued first (the HWDGE serializes; qkv k=0 needs both);
    # projext is not needed until the exponent phase.
    big = ctx.enter_context(tc.tile_pool(name="big", bufs=1))
    xt_s = []
    wt_s = []
    for k in range(KT):
        tx = big.tile([128, T], BF16, name=f"xt{k}", tag=f"xt{k}")
        nc.sync.dma_start(tx[:], xt[k * 128 : (k + 1) * 128, :])
        xt_s.append(tx)
        tw = big.tile([128, 3 * CP], BF16, name=f"wt{k}", tag=f"wt{k}")
        nc.scalar.dma_start(tw[:], wt[k * 128 : (k + 1) * 128, :])
        wt_s.append(tw)
        if k == 1:
            projext_s = const.tile(
                [2 * D, F], BF16, name="projext", tag="projext"
            )
            nc.scalar.dma_start(projext_s[:], projext)
    mask3_s = const.tile([S, 3 * S], BF16, name="mask3", tag="mask3")
    nc.sync.dma_start(mask3_s[:], mask3)
    ident_s = const.tile([128, 128], BF16, name="ident", tag="ident")
    nc.sync.dma_start(ident_s[:], ident)
    wpt_a = big.tile([128, C], BF16, name="wpt_a", tag="wpt_a")
    nc.scalar.dma_start(wpt_a[:], wpt[0:128, :])
    wpt_b = big.tile([D, C], BF16, name="wpt_b", tag="wpt_b")
    nc.scalar.dma_start(wpt_b[:], wpt[128:CP, :])

    # PSUM tags (8 banks): A x3 (qkv, transposes), Sc x2 (knat, scores,
    # yproj), Nd x2 (exponent, vdir, numden), St x1 (scan state)
    psum = ctx.enter_context(tc.tile_pool(name="psum", bufs=2, space="PSUM"))

    # persistent SBUF tensors
    stk = {}  # (kind, h): rows 0:64 = (q|k)^T head h, rows 64:128 squared
    qpTc = {}  # h -> [128, T]: rows 0:64 = Q'^T, rows 64:128 = K'^T
    kTp = {}  # h -> [64, T] base-0 copy of K'^T for the scores stationary
    for h in range(HPC):
        qpTc[h] = big.tile([128, T], BF16, name=f"qpTc{h}", tag=f"qpTc{h}")
        kTp[h] = big.tile([D, T], BF16, name=f"kTp{h}", tag=f"kTp{h}")
        for kind in "qk":
            stk[(kind, h)] = big.tile(
                [128, T], BF16, name=f"stk{kind}{h}", tag=f"stk{kind}{h}"
            )
    ones_t = big.tile([1, T], BF16, name="ones_t", tag="ones_t")
    nc.gpsimd.memset(ones_t[:], 1.0)
    epsrow = big.tile([1, HPC * E], BF16, name="epsrow", tag="epsrow")
    nc.gpsimd.memset(epsrow[:], 0.0)
    nc.gpsimd.memset(epsrow[:, D :: E], EPS_SCALED)
    vnat = {}  # chunk -> [S, 195] = [v0|1|v1|1|v2|1]
    for i in range(NCH):
        vnat[i] = big.tile([S, HPC * E], BF16, name=f"vn{i}", tag=f"vn{i}")
        nc.gpsimd.memset(vnat[i][:, D :: E], 1.0)
    knat = {}  # (h, quad) -> [S, 256]: chunks 4q..4q+3 in col quarters
    for h in range(HPC):
        for p in range(NCH // 4):
            knat[(h, p)] = big.tile(
                [S, 256], BF16, name=f"kn{h}_{p}", tag=f"kn{h}_{p}"
            )
    outT01 = big.tile([128, T], BF16, name="outT01", tag="outT01")
    outT2 = big.tile([D, T], BF16, name="outT2", tag="outT2")

    # state SBUF mirrors, rotating pair (chunk 0 skips the apply)
    ssb_tiles = [
        big.tile([F, HPC * E], BF16, name=f"ssb{j}", tag=f"ssb{j}")
        for j in range(2)
    ]

    vdir_chunks_done = [0]

    def emit_vdir(upto):
        # V in natural [t, d] layout via its own matmul; wt cols 384:576
        while vdir_chunks_done[0] < upto:
            i = vdir_chunks_done[0]
            pt = psum.tile([S, CP], FP32, name="vdp", tag="Nd")
            for k in range(KT):
                nc.tensor.matmul(
                    pt[:],
                    xt_s[k][:, i * S : (i + 1) * S],
                    wt_s[k][:, 384:576],
                    start=(k == 0),
                    stop=(k == KT - 1),
                )
            # one strided copy: psum [S,(3,64)] -> vnat cols (h*65..h*65+63)
            dst = vnat[i][:].rearrange("p (h c) -> p h c", h=HPC)[:, :, 0:D]
            src = pt[:].rearrange("p (h c) -> p h c", h=HPC)
            nc.vector.tensor_copy(dst, src)
            vdir_chunks_done[0] += 1

    # --- front-end, software-pipelined: head h+1's qkv runs while head
    # h's stk copies/squares and exponent/knat chain drains ---
    pts_t = {}

    def emit_qkv(h):
        pts = [
            psum.tile([128, 512], FP32, name=f"qkvp{h}{nh}", tag="A", bufs=3)
            for nh in range(2)
        ]
        pts_t[h] = pts
        for k in range(KT):
            lhs = wt_s[k][:, h * 128 : (h + 1) * 128]
            for nh in range(2):
                nc.tensor.matmul(
                    pts[nh][:],
                    lhs,
                    xt_s[k][:, nh * 512 : (nh + 1) * 512],
                    start=(k == 0),
                    stop=(k == KT - 1),
                )

    def emit_stk(h):
        pts = pts_t.pop(h)
        for nh in range(2):
            dst = slice(nh * 512, (nh + 1) * 512)
            for half, kind in enumerate("qk"):
                src = pts[nh][half * 64 : (half + 1) * 64, :]
                st = stk[(kind, h)]
                if (half + nh) == 0:
                    nc.scalar.activation(st[0:64, dst], src, COPY)
                else:
                    nc.vector.tensor_copy(st[0:64, dst], src)
                nc.vector.tensor_mul(
                    st[64:128, dst], st[0:64, dst], st[0:64, dst]
                )

    def emit_expknat(h):
        # q' and k' exponents share one [128,512] psum: ONE Exp per half
        for nh in range(2):
            cols = slice(nh * 512, (nh + 1) * 512)
            pt = psum.tile([128, 512], FP32, name="pp", tag="Nd")
            nc.tensor.matmul(
                pt[0:64, :], projext_s[:], stk[("q", h)][:, cols],
                start=True, stop=True,
            )
            nc.tensor.matmul(
                pt[64:128, :], projext_s[:], stk[("k", h)][:, cols],
                start=True, stop=True,
            )
            nc.scalar.activation(qpTc[h][:, cols], pt[:], EXP)
            # base-0 copy of the k half for the scores stationary (Pool
            # is idle here; PE lhsT/rhs must share a base partition)
            nc.gpsimd.tensor_copy(kTp[h][:, cols], qpTc[h][64:128, cols])
        # K' natural [t, f], chunk-quad psum -> one Exp per quad
        for p in range(NCH // 4):
            pt = psum.tile([S, 256], FP32, name="knp", tag="Sc")
            for j in range(4):
                i = 4 * p + j
                nc.tensor.matmul(
                    pt[:, j * 64 : (j + 1) * 64],
                    stk[("k", h)][:, i * S : (i + 1) * S],
                    projext_s[:],
                    start=True,
                    stop=True,
                )
            nc.scalar.activation(knat[(h, p)][:], pt[:], EXP)

    emit_qkv(0)
    emit_stk(0)
    emit_vdir(2)
    emit_qkv(1)
    emit_expknat(0)
    emit_stk(1)
    emit_vdir(4)
    emit_qkv(2)
    emit_expknat(1)
    emit_stk(2)
    emit_vdir(6)
    emit_expknat(2)
    emit_vdir(NCH)

    # --- scan: chunk-outer, head-inner ---
    sb = ctx.enter_context(tc.tile_pool(name="scan_sb", bufs=3))
    och_pair = {}
    och2 = {}
    for i in range(NCH):
        och_pair[i] = sb.tile(
            [S, 128], BF16, name=f"ochp{i}", tag="ochp", bufs=NCH
        )
        och2[i] = sb.tile([S, D], BF16, name=f"och2_{i}", tag="och2", bufs=NCH)

    # per-chunk update delta lives in PSUM; the running state accumulates
    # in SBUF (re-opening a stopped PSUM accumulation group loses data)

    stm_t = {}
    ndp_t = {}

    def emit_scores(j):
        cj = slice(j * S, (j + 1) * S)
        sc3 = psum.tile([S, HPC * S], FP32, name="sc3", tag="Sc")
        for h in range(HPC):
            nc.tensor.matmul(
                sc3[:, h * S : (h + 1) * S],
                kTp[h][:, cj],
                qpTc[h][0:64, cj],
                start=True,
                stop=True,
            )
        stm = sb.tile([S, HPC * S], BF16, name="stm", tag="stm", bufs=3)
        if j % 2 == 0:
            nc.vector.tensor_mul(stm[:], sc3[:], mask3_s[:])
        else:
            scb = sb.tile([S, HPC * S], BF16, name="scb", tag="scb")
            nc.scalar.activation(scb[:], sc3[:], COPY)
            nc.gpsimd.tensor_mul(stm[:], scb[:], mask3_s[:])
        stm_t[j] = stm

    def emit_division(j):
        """Division for chunk j (lagged one chunk off the state chain)."""
        ndp = ndp_t.pop(j)
        dinv3 = sb.tile([S, HPC], FP32, name="dinv3", tag="dinv3")
        nc.vector.reciprocal(dinv3[:], ndp[:, D :: E])
        for h in range(HPC):
            och = och_pair[j][:, h * D : (h + 1) * D] if h < 2 else och2[j][:]
            if (h + j) % 2 == 0:
                nc.scalar.activation(
                    och, ndp[:, h * E : h * E + D], COPY,
                    scale=dinv3[:, h : h + 1],
                )
            else:
                nc.vector.tensor_scalar_mul(
                    och, ndp[:, h * E : h * E + D], dinv3[:, h : h + 1]
                )

    def emit_tail(j):
        """Transposes, yproj, output copy + DMA for chunk j (post-scan)."""
        cj = slice(j * S, (j + 1) * S)
        tp = psum.tile([128, S], BF16, name="tp", tag="A", bufs=3)
        nc.tensor.transpose(tp[:], och_pair[j][:], ident_s[:])
        tp2 = psum.tile([D, S], BF16, name="tp2", tag="A", bufs=3)
        nc.tensor.transpose(tp2[:], och2[j][:], ident_s[:])
        nc.vector.tensor_copy(outT01[:, cj], tp[:])
        nc.scalar.activation(outT2[:, cj], tp2[:], COPY)
        # yproj: stationary = transposed och chunk, moving = wpt
        yp1 = psum.tile([S, 512], FP32, name="yp1", tag="Sc")
        yp2 = psum.tile([S, 256], FP32, name="yp2", tag="Sc")
        for yp, ccols in ((yp1, slice(0, 512)), (yp2, slice(512, 768))):
            nc.tensor.matmul(
                yp[:], outT01[:, cj], wpt_a[:, ccols], start=True, stop=False
            )
            nc.tensor.matmul(
                yp[:], outT2[:, cj], wpt_b[:, ccols], start=False, stop=True
            )
        yrow = sb.tile([S, C], BF16, name="yrow", tag="yrow")
        if j % 2 == 0:
            nc.scalar.activation(yrow[:, 0:512], yp1[:], COPY)
            nc.vector.tensor_copy(yrow[:, 512:768], yp2[:])
        else:
            nc.vector.tensor_copy(yrow[:, 0:512], yp1[:])
            nc.scalar.activation(yrow[:, 512:768], yp2[:], COPY)
        dma_eng = nc.sync if j % 2 == 0 else nc.scalar
        dma_eng.dma_start(y[j * S : (j + 1) * S, :], yrow[:])

    emit_scores(0)
    emit_scores(1)
    for i in range(NCH):
        ci = slice(i * S, (i + 1) * S)
        stm = stm_t.pop(i)
        ssb_prev = ssb_tiles[(i - 1) % 2]
        ndp = psum.tile([S, HPC * E], FP32, name="ndp", tag="Nd")
        ndp_t[i] = ndp
        delta = psum.tile([F, HPC * E], FP32, name="delta", tag="St", bufs=1)
        for h in range(HPC):
            he = slice(h * E, (h + 1) * E)
            nc.tensor.matmul(
                ndp[:, he], stm[:, h * S : (h + 1) * S], vnat[i][:, he],
                start=True, stop=False,
            )
            if i > 0:
                nc.tensor.matmul(
                    ndp[:, he], qpTc[h][0:64, ci], ssb_prev[:, he],
                    start=False, stop=False,
                )
            # EPS lands on the den column via a K=1 matmul
            nc.tensor.matmul(
                ndp[:, he], ones_t[:, ci], epsrow[:, he],
                start=False, stop=True,
            )
            nc.tensor.matmul(
                delta[:, he],
                knat[(h, i // 4)][:, (i % 4) * 64 : (i % 4 + 1) * 64],
                vnat[i][:, he],
                start=True,
                stop=True,
            )
        if i < NCH - 1:
            if i < NCH - 2:
                emit_scores(i + 2)
            ssb_cur = ssb_tiles[i % 2]
            if i == 0:
                nc.vector.tensor_copy(ssb_cur[:], delta[:])
            else:
                nc.vector.tensor_add(ssb_cur[:], delta[:], ssb_prev[:])
        if i > 0:
            emit_division(i - 1)
    emit_division(NCH - 1)
    for j in range(NCH):
        emit_tail(j)


_PROGRAM = None


def _get_program():
    global _PROGRAM
    if _PROGRAM is None:
        _PROGRAM = build_program()
    return _PROGRAM


def make_core_inputs(x, W_attn, W_proj, proj, core):
    import ml_dtypes

    bf = ml_dtypes.bfloat16
    b, hg = divmod(core, 4)
    heads = list(range(HPC * hg, HPC * (hg + 1)))
    rows = []
    for h in heads:  # (q_h | k_h) pairs, then the v block
        rows.extend(range(h * D, (h + 1) * D))
        rows.extend(range(C + h * D, C + (h + 1) * D))
    for h in heads:
        rows.extend(range(2 * C + h * D, 2 * C + (h + 1) * D))
    projext = np.concatenate(
        [proj.astype(np.float32), np.full((D, F), -0.5, np.float32)], axis=0
    )
    mask = np.triu(np.ones((S, S), np.float32))
    return {
        "xt": np.ascontiguousarray(x[b].T).astype(bf),
        "wt": np.ascontiguousarray(W_attn[rows, :].T).astype(bf),
        "wpt": np.ascontiguousarray(
            W_proj[:, CP * hg : CP * (hg + 1)].T
        ).astype(bf),
        "projext": projext.astype(bf),
        "mask3": np.tile(mask, (1, HPC)).astype(bf),
        "ident": np.eye(128, dtype=np.float32).astype(bf),
    }


def kernel(x, W_attn, W_proj, proj):
    nc = _get_program()
    in_maps = [
        make_core_inputs(x, W_attn, W_proj, proj, core) for core in range(N_CORES)
    ]
    res = run_bass_kernel_spmd(nc, in_maps, list(range(N_CORES)))
    out = np.empty((B, T, C), np.float32)
    for b in range(B):
        acc = res.results[4 * b]["y"].astype(np.float32)
        for g in range(1, 4):
            acc = acc + res.results[4 * b + g]["y"].astype(np.float32)
        out[b] = acc
    return out


# revision 3
# speedup vs baseline: 1.0493x; 1.0493x over previous
"""Causal Performer attention on 8 trn2 NeuronCores (bf16, pipelined).

Sharding: core c handles batch b = c // 4 and head-group hg = c % 4
(3 of the 12 heads). Each core:
  1. computes the qkv projection for its 3 heads (576 of 2304 rows),
  2. runs the causal linear-attention scan in chunked form: intra-chunk
     masked (K'Q') scores plus an inter-chunk (F,D) running state,
     updated/applied every 128-token chunk with the 3 heads interleaved
     inside each chunk step,
  3. projects its 192 channels into a partial y in [t, c]-major layout
     whose 128-row chunks stream out over DMA as they are computed.
The host upcasts and sums the four partial bf16 (T, C) projections per
batch (pure gather/unshard work).

All matmul operands are bf16; PSUM accumulation stays fp32 (measured
rel err vs the fp32 reference: 1.03e-2, gate 2e-2). bf16 runs every
matmul at 1 cycle/row (small-N fp32 matmuls cost 4) and halves the
input DMA stream, which paces the front-end.

Key structures (emission order is the per-engine execution order):
  - q'^T/K'^T = exp([P; -0.5*ones]^T [q; q^2]) via one stacked matmul
    per half; q' and k' share one [128,512] PSUM tile (q rows 0:64,
    k rows 64:128) so a single ACT Exp pass serves both; Pool copies
    the k half to a base-0 tile for the scores stationary (PE operands
    must share a base partition). Squares run on DVE from bf16 rows.
  - The front-end is software-pipelined: head h+1's qkv matmuls run
    while head h's stk copies / squares / Exp chain drains; vdir
    (V in natural [t,d] layout) fills the remaining PE gaps.
  - The scan state accumulates in SBUF: each chunk's update lands in a
    fresh PSUM delta tile (3 per-head groups that open and close within
    the chunk -- re-opening a stopped PSUM accumulation group loses its
    contents) and one DVE tensor_add folds it into the running bf16
    state while producing the next apply operand.
  - EPS rides on the numerator/denominator tile via K=1 matmuls
    (ones[1,chunk] x [0..0,EPS] per head) so the division needs no
    extra scalar adds; scores+mask are pipelined two chunks ahead
    (mask multiply alternates DVE direct and ACT-copy + Pool-mul);
    division lags one chunk, off the state chain.
  - V for the 3 heads lives interleaved [v0|1|v1|1|v2|1] in one [S,195]
    tile per chunk (one strided DVE copy; ones columns memset once);
    the 3 heads' scores / numden share PSUM tiles (one mask op, one
    strided reciprocal).
  - Per chunk, och is transposed ([t,ch] -> [ch,t]) and used as the
    yproj stationary with wpt moving, yielding y[t-chunk, 768] directly;
    the row-chunk DMAs out immediately, so there is no output tail.
"""

import numpy as np

import concourse.bacc as bacc
import concourse.bass as bass
import concourse.mybir as mybir
from concourse import tile
from concourse.bass_utils import run_bass_kernel_spmd

B, T, C = 2, 1024, 768
H, D, F = 12, 64, 64
HPC = 3  # heads per core
S = 128  # scan chunk length
NCH = T // S  # 8 chunks
CP = HPC * D  # 192 channels per core
N_CORES = 8
KT = C // 128  # 6 contraction tiles for the qkv matmul
EPS_SCALED = float(F) * 1e-6  # compensates dropping 1/sqrt(F) on q', k'
E = D + 1  # 65: [V | 1] block width

FP32 = mybir.dt.float32
BF16 = mybir.dt.bfloat16
EXP = mybir.ActivationFunctionType.Exp
COPY = mybir.ActivationFunctionType.Copy


def build_program(n_iters=1):
    nc = bacc.Bacc(
        "TRN2", target_bir_lowering=False, debug=False, num_devices=N_CORES
    )
    xt = nc.dram_tensor("xt", [C, T], BF16, kind="ExternalInput").ap()
    wt = nc.dram_tensor("wt", [C, 3 * CP], BF16, kind="ExternalInput").ap()
    wpt = nc.dram_tensor("wpt", [CP, C], BF16, kind="ExternalInput").ap()
    projext = nc.dram_tensor("projext", [2 * D, F], BF16, kind="ExternalInput").ap()
    mask3 = nc.dram_tensor("mask3", [S, 3 * S], BF16, kind="ExternalInput").ap()
    ident = nc.dram_tensor("ident", [128, 128], BF16, kind="ExternalInput").ap()
    y = nc.dram_tensor("y", [T, C], BF16, kind="ExternalOutput").ap()

    from contextlib import ExitStack

    with tile.TileContext(nc) as tc:
        for _ in range(n_iters):
            with ExitStack() as ctx:
                _body(ctx, tc, xt, wt, wpt, projext, mask3, ident, y)
    nc.compile()
    return nc


def _body(ctx, tc, xt, wt, wpt, projext, mask3, ident, y):
    nc = tc.nc

    const = ctx.enter_context(tc.tile_pool(name="const", bufs=1))

    # xt + late consts on the SP queue; wt/wpt on the ACT queue. xt0 and
    # wt0 are issued first (the HWDGE serializes; qkv k=0 needs both);
    # projext is not needed until the exponent phase.
    big = ctx.enter_context(tc.tile_pool(name="big", bufs=1))
    xt_s = []
    wt_s = []
    for k in range(KT):
        tx = big.tile([128, T], BF16, name=f"xt{k}", tag=f"xt{k}")
        nc.sync.dma_start(tx[:], xt[k * 128 : (k + 1) * 128, :])
        xt_s.append(tx)
        tw = big.tile([128, 3 * CP], BF16, name=f"wt{k}", tag=f"wt{k}")
        nc.scalar.dma_start(tw[:], wt[k * 128 : (k + 1) * 128, :])
        wt_s.append(tw)
        if k == 1:
            projext_s = const.tile(
                [2 * D, F], BF16, name="projext", tag="projext"
            )
            nc.scalar.dma_start(projext_s[:], projext)
    mask3_s = const.tile([S, 3 * S], BF16, name="mask3", tag="mask3")
    nc.sync.dma_start(mask3_s[:], mask3)
    ident_s = const.tile([128, 128], BF16, name="ident", tag="ident")
    nc.sync.dma_start(ident_s[:], ident)
    wpt_a = big.tile([128, C], BF16, name="wpt_a", tag="wpt_a")
    nc.scalar.dma_start(wpt_a[:], wpt[0:128, :])
    wpt_b = big.tile([D, C], BF16, name="wpt_b", tag="wpt_b")
    nc.scalar.dma_start(wpt_b[:], wpt[128:CP, :])

    # PSUM tags (8 banks): A x3 (qkv, transposes), Sc x2 (knat, scores,
    # yproj), Nd x2 (exponent, vdir, numden), St x1 (scan state)
    psum = ctx.enter_context(tc.tile_pool(name="psum", bufs=2, space="PSUM"))

    # persistent SBUF tensors
    stk = {}  # (kind, h): rows 0:64 = (q|k)^T head h, rows 64:128 squared
    qpTc = {}  # h -> [128, T]: rows 0:64 = Q'^T, rows 64:128 = K'^T
    kTp = {}  # h -> [64, T] base-0 copy of K'^T for the scores stationary
    for h in range(HPC):
        qpTc[h] = big.tile([128, T], BF16, name=f"qpTc{h}", tag=f"qpTc{h}")
        kTp[h] = big.tile([D, T], BF16, name=f"kTp{h}", tag=f"kTp{h}")
        for kind in "qk":
            stk[(kind, h)] = big.tile(
                [128, T], BF16, name=f"stk{kind}{h}", tag=f"stk{kind}{h}"
            )
    ones_t = big.tile([1, T], BF16, name="ones_t", tag="ones_t")
    nc.gpsimd.memset(ones_t[:], 1.0)
    epsrow = big.tile([1, HPC * E], BF16, name="epsrow", tag="epsrow")
    nc.gpsimd.memset(epsrow[:], 0.0)
    nc.gpsimd.memset(epsrow[:, D :: E], EPS_SCALED)
    vnat = {}  # chunk -> [S, 195] = [v0|1|v1|1|v2|1]
    for i in range(NCH):
        vnat[i] = big.tile([S, HPC * E], BF16, name=f"vn{i}", tag=f"vn{i}")
        nc.gpsimd.memset(vnat[i][:, D :: E], 1.0)
    knat = {}  # (h, quad) -> [S, 256]: chunks 4q..4q+3 in col quarters
    for h in range(HPC):
        for p in range(NCH // 4):
            knat[(h, p)] = big.tile(
                [S, 256], BF16, name=f"kn{h}_{p}", tag=f"kn{h}_{p}"
            )
    outT01 = big.tile([128, T], BF16, name="outT01", tag="outT01")
    outT2 = big.tile([D, T], BF16, name="outT2", tag="outT2")

    # state SBUF mirrors, rotating pair (chunk 0 skips the apply)
    ssb_tiles = [
        big.tile([F, HPC * E], BF16, name=f"ssb{j}", tag=f"ssb{j}")
        for j in range(2)
    ]

    vdir_chunks_done = [0]

    def emit_vdir(upto):
        # V in natural [t, d] layout via its own matmul; wt cols 384:576
        while vdir_chunks_done[0] < upto:
            i = vdir_chunks_done[0]
            pt = psum.tile([S, CP], FP32, name="vdp", tag="Nd")
            for k in range(KT):
                nc.tensor.matmul(
                    pt[:],
                    xt_s[k][:, i * S : (i + 1) * S],
                    wt_s[k][:, 384:576],
                    start=(k == 0),
                    stop=(k == KT - 1),
                )
            # one strided copy: psum [S,(3,64)] -> vnat cols (h*65..h*65+63)
            dst = vnat[i][:].rearrange("p (h c) -> p h c", h=HPC)[:, :, 0:D]
            src = pt[:].rearrange("p (h c) -> p h c", h=HPC)
            if i % 2 == 0:
                nc.vector.tensor_copy(dst, src)
            else:
                nc.scalar.activation(dst, src, COPY)
            vdir_chunks_done[0] += 1

    # --- front-end, software-pipelined: head h+1's qkv runs while head
    # h's stk copies/squares and exponent/knat chain drains ---
    pts_t = {}

    def emit_qkv(h):
        pts = [
            psum.tile([128, 512], FP32, name=f"qkvp{h}{nh}", tag="A", bufs=3)
            for nh in range(2)
        ]
        pts_t[h] = pts
        for k in range(KT):
            lhs = wt_s[k][:, h * 128 : (h + 1) * 128]
            for nh in range(2):
                nc.tensor.matmul(
                    pts[nh][:],
                    lhs,
                    xt_s[k][:, nh * 512 : (nh + 1) * 512],
                    start=(k == 0),
                    stop=(k == KT - 1),
                )

    def emit_stk(h):
        pts = pts_t.pop(h)
        for nh in range(2):
            dst = slice(nh * 512, (nh + 1) * 512)
            for half, kind in enumerate("qk"):
                src = pts[nh][half * 64 : (half + 1) * 64, :]
                st = stk[(kind, h)]
                if (half + nh) % 2 == 0:
                    nc.scalar.activation(st[0:64, dst], src, COPY)
                else:
                    nc.vector.tensor_copy(st[0:64, dst], src)
                nc.vector.tensor_mul(
                    st[64:128, dst], st[0:64, dst], st[0:64, dst]
                )

    def emit_expknat(h):
        # q' and k' exponents share one [128,512] psum: ONE Exp per half
        for nh in range(2):
            cols = slice(nh * 512, (nh + 1) * 512)
            pt = psum.tile([128, 512], FP32, name="pp", tag="Nd")
            nc.tensor.matmul(
                pt[0:64, :], projext_s[:], stk[("q", h)][:, cols],
                start=True, stop=True,
            )
            nc.tensor.matmul(
                pt[64:128, :], projext_s[:], stk[("k", h)][:, cols],
                start=True, stop=True,
            )
            nc.scalar.activation(qpTc[h][:, cols], pt[:], EXP)
            # base-0 copy of the k half for the scores stationary (Pool
            # is idle here; PE lhsT/rhs must share a base partition)
            nc.gpsimd.tensor_copy(kTp[h][:, cols], qpTc[h][64:128, cols])
        # K' natural [t, f], chunk-quad psum -> one Exp per quad
        for p in range(NCH // 4):
            pt = psum.tile([S, 256], FP32, name="knp", tag="Sc")
            for j in range(4):
                i = 4 * p + j
                nc.tensor.matmul(
                    pt[:, j * 64 : (j + 1) * 64],
                    stk[("k", h)][:, i * S : (i + 1) * S],
                    projext_s[:],
                    start=True,
                    stop=True,
                )
            nc.scalar.activation(knat[(h, p)][:], pt[:], EXP)

    emit_qkv(0)
    emit_stk(0)
    emit_vdir(2)
    emit_qkv(1)
    emit_expknat(0)
    emit_stk(1)
    emit_vdir(4)
    emit_qkv(2)
    emit_expknat(1)
    emit_stk(2)
    emit_vdir(6)
    emit_expknat(2)
    emit_vdir(NCH)

    # --- scan: chunk-outer, head-inner ---
    sb = ctx.enter_context(tc.tile_pool(name="scan_sb", bufs=3))
    och_pair = {}
    och2 = {}
    for i in range(NCH):
        och_pair[i] = sb.tile(
            [S, 128], BF16, name=f"ochp{i}", tag="ochp", bufs=NCH
        )
        och2[i] = sb.tile([S, D], BF16, name=f"och2_{i}", tag="och2", bufs=NCH)

    # per-chunk update delta lives in PSUM; the running state accumulates
    # in SBUF (re-opening a stopped PSUM accumulation group loses data)

    stm_t = {}
    ndp_t = {}

    def emit_scores(j):
        cj = slice(j * S, (j + 1) * S)
        sc3 = psum.tile([S, HPC * S], FP32, name="sc3", tag="Sc")
        for h in range(HPC):
            nc.tensor.matmul(
                sc3[:, h * S : (h + 1) * S],
                kTp[h][:, cj],
                qpTc[h][0:64, cj],
                start=True,
                stop=True,
            )
        stm = sb.tile([S, HPC * S], BF16, name="stm", tag="stm", bufs=3)
        if j % 2 == 0:
            nc.vector.tensor_mul(stm[:], sc3[:], mask3_s[:])
        else:
            scb = sb.tile([S, HPC * S], BF16, name="scb", tag="scb")
            nc.scalar.activation(scb[:], sc3[:], COPY)
            nc.gpsimd.tensor_mul(stm[:], scb[:], mask3_s[:])
        stm_t[j] = stm

    def emit_division(j):
        """Division for chunk j (lagged one chunk off the state chain)."""
        ndp = ndp_t.pop(j)
        dinv3 = sb.tile([S, HPC], FP32, name="dinv3", tag="dinv3")
        nc.vector.reciprocal(dinv3[:], ndp[:, D :: E])
        for h in range(HPC):
            och = och_pair[j][:, h * D : (h + 1) * D] if h < 2 else och2[j][:]
            if (h + j) % 2 == 0:
                nc.scalar.activation(
                    och, ndp[:, h * E : h * E + D], COPY,
                    scale=dinv3[:, h : h + 1],
                )
            else:
                nc.vector.tensor_scalar_mul(
                    och, ndp[:, h * E : h * E + D], dinv3[:, h : h + 1]
                )

    def emit_tail(j):
        """Transposes, yproj, output copy + DMA for chunk j (post-scan)."""
        cj = slice(j * S, (j + 1) * S)
        tp = psum.tile([128, S], BF16, name="tp", tag="A", bufs=3)
        nc.tensor.transpose(tp[:], och_pair[j][:], ident_s[:])
        tp2 = psum.tile([D, S], BF16, name="tp2", tag="A", bufs=3)
        nc.tensor.transpose(tp2[:], och2[j][:], ident_s[:])
        nc.vector.tensor_copy(outT01[:, cj], tp[:])
        nc.scalar.activation(outT2[:, cj], tp2[:], COPY)
        # yproj: stationary = transposed och chunk, moving = wpt
        yp1 = psum.tile([S, 512], FP32, name="yp1", tag="Sc")
        yp2 = psum.tile([S, 256], FP32, name="yp2", tag="Sc")
        for yp, ccols in ((yp1, slice(0, 512)), (yp2, slice(512, 768))):
            nc.tensor.matmul(
                yp[:], outT01[:, cj], wpt_a[:, ccols], start=True, stop=False
            )
            nc.tensor.matmul(
                yp[:], outT2[:, cj], wpt_b[:, ccols], start=False, stop=True
            )
        yrow = sb.tile([S, C], BF16, name="yrow", tag="yrow")
        if j % 2 == 0:
            nc.scalar.activation(yrow[:, 0:512], yp1[:], COPY)
            nc.vector.tensor_copy(yrow[:, 512:768], yp2[:])
        else:
            nc.vector.tensor_copy(yrow[:, 0:512], yp1[:])
            nc.scalar.activation(yrow[:, 512:768], yp2[:], COPY)
        dma_eng = nc.sync if j % 2 == 0 else nc.scalar
        dma_eng.dma_start(y[j * S : (j + 1) * S, :], yrow[:])

    emit_scores(0)
    emit_scores(1)
    for i in range(NCH):
        ci = slice(i * S, (i + 1) * S)
        stm = stm_t.pop(i)
        ssb_prev = ssb_tiles[(i - 1) % 2]
        ndp = psum.tile([S, HPC * E], FP32, name="ndp", tag="Nd")
        ndp_t[i] = ndp
        delta = psum.tile([F, HPC * E], FP32, name="delta", tag="St", bufs=1)
        for h in range(HPC):
            he = slice(h * E, (h + 1) * E)
            nc.tensor.matmul(
                ndp[:, he], stm[:, h * S : (h + 1) * S], vnat[i][:, he],
                start=True, stop=False,
            )
            if i > 0:
                nc.tensor.matmul(
                    ndp[:, he], qpTc[h][0:64, ci], ssb_prev[:, he],
                    start=False, stop=False,
                )
            # EPS lands on the den column via a K=1 matmul
            nc.tensor.matmul(
                ndp[:, he], ones_t[:, ci], epsrow[:, he],
                start=False, stop=True,
            )
            nc.tensor.matmul(
                delta[:, he],
                knat[(h, i // 4)][:, (i % 4) * 64 : (i % 4 + 1) * 64],
                vnat[i][:, he],
                start=True,
                stop=True,
            )
        if i < NCH - 1:
            if i < NCH - 2:
                emit_scores(i + 2)
            ssb_cur = ssb_tiles[i % 2]
            if i == 0:
                nc.vector.tensor_copy(ssb_cur[:], delta[:])
            else:
                nc.vector.tensor_add(ssb_cur[:], delta[:], ssb_prev[:])
        if i > 0:
            emit_division(i - 1)
    emit_division(NCH - 1)
    for j in range(NCH):
        emit_tail(j)


_PROGRAM = None


def _get_program():
    global _PROGRAM
    if _PROGRAM is None:
        _PROGRAM = build_program()
    return _PROGRAM


def make_core_inputs(x, W_attn, W_proj, proj, core):
    import ml_dtypes

    bf = ml_dtypes.bfloat16
    b, hg = divmod(core, 4)
    heads = list(range(HPC * hg, HPC * (hg + 1)))
    rows = []
    for h in heads:  # (q_h | k_h) pairs, then the v block
        rows.extend(range(h * D, (h + 1) * D))
        rows.extend(range(C + h * D, C + (h + 1) * D))
    for h in heads:
        rows.extend(range(2 * C + h * D, 2 * C + (h + 1) * D))
    projext = np.concatenate(
        [proj.astype(np.float32), np.full((D, F), -0.5, np.float32)], axis=0
    )
    mask = np.triu(np.ones((S, S), np.float32))
    return {
        "xt": np.ascontiguousarray(x[b].T).astype(bf),
        "wt": np.ascontiguousarray(W_attn[rows, :].T).astype(bf),
        "wpt": np.ascontiguousarray(
            W_proj[:, CP * hg : CP * (hg + 1)].T
        ).astype(bf),
        "projext": projext.astype(bf),
        "mask3": np.tile(mask, (1, HPC)).astype(bf),
        "ident": np.eye(128, dtype=np.float32).astype(bf),
    }


def kernel(x, W_attn, W_proj, proj):
    nc = _get_program()
    in_maps = [
        make_core_inputs(x, W_attn, W_proj, proj, core) for core in range(N_CORES)
    ]
    res = run_bass_kernel_spmd(nc, in_maps, list(range(N_CORES)))
    out = np.empty((B, T, C), np.float32)
    for b in range(B):
        acc = res.results[4 * b]["y"].astype(np.float32)
        for g in range(1, 4):
            acc = acc + res.results[4 * b + g]["y"].astype(np.float32)
        out[b] = acc
    return out
